# revision 6
# baseline (speedup 1.0000x reference)
"""EMA head kernel for Trainium2 (Bass/Tile), 8 NeuronCores.

Problem: alpha = clip(sigmoid(MLP(feat)), 0.01, 0.99) per (t, b);
         y[0] = r[0]; y[t] = (1-alpha[t])*y[t-1] + alpha[t]*r[t].

Sharding: time dim T=4096 split into 8 slabs of 512 (all B=256 per core).
Each core computes, for its slab, the local affine-scan pieces
    z[t] = A[t]*z[t-1] + Bv[t]   (z[-1] = 0),   A = 1-alpha, Bv = alpha*r
    P[t] = A[t]*P[t-1]           (P[-1] = 1)
and the host stitches slabs with   y = z + P * carry,  carry' = y[-1].
carry_0 = r[0] reproduces y[0] = r[0] exactly: a*r + (1-a)*r = r.

v7: pipelined per-chunk tail (chained scans); feat is cast to fp16 AND transposed to [f, t*b] on the host, so the
device does plain contiguous HWDGE DMA at full HBM rate and zero on-chip
transposes.  All small constants are host-replicated and loaded as plain
contiguous HWDGE transfers (no SWDGE broadcast descriptors, which stall
the SDMA engines for ~100us).  b1 is folded into the PSUM via a rank-1
PE matmul that initializes each h-bank (start=True), with the per-t-step
matmuls accumulating on top (start=False).  Block epilogue: ACT relu
(PSUM->SBUF), mul by w2 (alternating DVE/GpSimd), DVE reduce ->
alpha_pre [128 b, t]; then sigmoid+clip and tensor_tensor_scan for z/P.
"""

import numpy as np

T, B, FEAT, HID = 4096, 256, 128, 16
NCORES = 8
TLOC = T // NCORES  # 512
NH = 2              # batch halves of 128
CH = 16384          # (t,b) columns per feat chunk (64 t-steps, 4 MB fp16)
NCHUNK = TLOC * B // CH  # 8

_CACHE = {}


def _build_program():
    import concourse.bacc as bacc
    import concourse.bass as bass
    import concourse.tile as tile
    from concourse import mybir

    fp32 = mybir.dt.float32
    fp16 = mybir.dt.float16
    AF = mybir.ActivationFunctionType
    OP = mybir.AluOpType

    nc = bacc.Bacc("TRN2", target_bir_lowering=False, debug=False,
                   num_devices=NCORES)

    feat_d = nc.dram_tensor("feat", [FEAT, TLOC * B], fp16, kind="ExternalInput")
    rt_d = nc.dram_tensor("rt", [NH, 128, TLOC], fp32, kind="ExternalInput")
    w1_d = nc.dram_tensor("w1", [FEAT, HID], fp16, kind="ExternalInput")
    b1r_d = nc.dram_tensor("b1r", [1, 32 * HID], fp16, kind="ExternalInput")
    w2rep_d = nc.dram_tensor("w2rep", [128, 32 * HID], fp32, kind="ExternalInput")
    b2col_d = nc.dram_tensor("b2col", [128, 1], fp32, kind="ExternalInput")
    z_d = nc.dram_tensor("z", [NH, 128, TLOC], fp32, kind="ExternalOutput")
    p_d = nc.dram_tensor("p", [NH, 128, TLOC], fp32, kind="ExternalOutput")

    with tile.TileContext(nc) as tc:
        with (
            tc.tile_pool(name="singles", bufs=1) as singles,
            tc.tile_pool(name="featin", bufs=3) as featin,
            tc.tile_pool(name="hps", bufs=2, space="PSUM") as hps,
            tc.tile_pool(name="hwork", bufs=2) as hwork,
        ):
            # ------------- constants / small inputs (all HWDGE) -------------
            w1_sb = singles.tile([128, HID], fp16)
            nc.scalar.dma_start(w1_sb, w1_d[:, :])
            b1row = singles.tile([1, 32 * HID], fp16)
            nc.scalar.dma_start(b1row, b1r_d[:, :])
            ones1 = singles.tile([1, 128], fp16)
            nc.vector.memset(ones1, 1.0)
            w2rep = singles.tile([128, 32, HID], fp32)
            nc.scalar.dma_start(
                w2rep, w2rep_d[:, :].rearrange("p (t h) -> p t h", h=HID))
            b2col = singles.tile([128, 1], fp32)
            nc.scalar.dma_start(b2col, b2col_d[:, :])
            ones_sb = singles.tile([128, TLOC], fp32)
            nc.vector.memset(ones_sb, 1.0)

            rT = [singles.tile([128, TLOC], fp32, tag=f"rT{h}", name=f"rT{h}")
                  for h in range(NH)]
            for h in range(NH):
                nc.scalar.dma_start(rT[h], rt_d[h])

            # per-half alpha_pre accumulators [128 b, t] + tail tiles
            apre = [singles.tile([128, TLOC], fp32, tag=f"apre{h}", name=f"apre{h}")
                    for h in range(NH)]
            alpha = [singles.tile([128, TLOC], fp32, tag=f"alpha{h}",
                                  name=f"alpha{h}") for h in range(NH)]
            A_sb = [singles.tile([128, TLOC], fp32, tag=f"A{h}",
                                 name=f"A{h}") for h in range(NH)]
            Bv = [singles.tile([128, TLOC], fp32, tag=f"Bv{h}",
                               name=f"Bv{h}") for h in range(NH)]
            z_sb = [singles.tile([128, TLOC], fp32, tag=f"z{h}",
                                 name=f"z{h}") for h in range(NH)]
            p_sb = [singles.tile([128, TLOC], fp32, tag=f"p{h}",
                                 name=f"p{h}") for h in range(NH)]

            # ---------------- main feat pipeline ----------------
            TCH = CH // B  # t-steps per chunk (64)
            mul_parity = 0
            for k in range(NCHUNK):
                ft = featin.tile([128, CH], fp16, tag="ft")
                nc.sync.dma_start(ft, feat_d[:, k * CH:(k + 1) * CH])
                # 32-t blocks within this chunk
                for blk in range(TCH // 32):
                    hbank = [hps.tile([128, 32, HID], fp32, tag=f"h{h}",
                                      name=f"hbank{h}")
                             for h in range(NH)]
                    for h in range(NH):
                        # rank-1 bias: psum[:, t, hid] = b1[hid]
                        nc.tensor.matmul(hbank[h][:, :, :], ones1, b1row,
                                         start=True, stop=False,
                                         skip_group_check=True)
                    for tt in range(32):
                        col = (blk * 32 + tt) * B
                        for h in range(NH):
                            nc.tensor.matmul(
                                hbank[h][:, tt, :],
                                ft[:, col + h * 128:col + (h + 1) * 128],
                                w1_sb, start=False, stop=True,
                                skip_group_check=True)
                    t0 = k * TCH + blk * 32
                    for h in range(NH):
                        hrelu = hwork.tile([128, 32, HID], fp32, tag="hrelu")
                        nc.scalar.activation(hrelu, hbank[h], AF.Relu)
                        hw = hwork.tile([128, 32, HID], fp32, tag="hw")
                        if mul_parity == 0:
                            nc.vector.tensor_mul(hw, hrelu, w2rep)
                        else:
                            nc.gpsimd.tensor_mul(hw, hrelu, w2rep)
                        mul_parity ^= 1
                        nc.vector.tensor_reduce(
                            apre[h][:, t0:t0 + 32],
                            hw, axis=mybir.AxisListType.X, op=OP.add)

                # pipelined tail for this chunk's 64 t-steps (chained scans)
                s0 = k * TCH
                sl = slice(s0, s0 + TCH)
                for h in range(NH):
                    eng = nc.vector if (k + h) % 2 == 0 else nc.gpsimd
                    nc.scalar.activation(alpha[h][:, sl], apre[h][:, sl],
                                         AF.Sigmoid, bias=b2col)
                    eng.tensor_scalar(alpha[h][:, sl], alpha[h][:, sl],
                                      0.01, 0.99, op0=OP.max, op1=OP.min)
                    eng.tensor_scalar(A_sb[h][:, sl], alpha[h][:, sl],
                                      -1.0, 1.0, op0=OP.mult, op1=OP.add)
                    eng.tensor_mul(Bv[h][:, sl], alpha[h][:, sl], rT[h][:, sl])
                    nc.vector.tensor_tensor_scan(
                        z_sb[h][:, sl], A_sb[h][:, sl], Bv[h][:, sl],
                        0.0 if k == 0 else z_sb[h][:, s0 - 1:s0],
                        op0=OP.mult, op1=OP.add)
                    nc.vector.tensor_tensor_scan(
                        p_sb[h][:, sl], A_sb[h][:, sl], ones_sb[:, sl],
                        1.0 if k == 0 else p_sb[h][:, s0 - 1:s0],
                        op0=OP.mult, op1=OP.mult)

            for h in range(NH):
                nc.scalar.dma_start(z_d[h], z_sb[h])
                nc.scalar.dma_start(p_d[h], p_sb[h])

    nc.finalize()
    return nc


def _get_program():
    if "nc" not in _CACHE:
        _CACHE["nc"] = _build_program()
    return _CACHE["nc"]


def _host_in_maps(r, feat, W1, b1, W2, b2):
    W1 = np.asarray(W1, dtype=np.float16)
    b1 = np.asarray(b1, dtype=np.float32).reshape(HID)
    W2 = np.asarray(W2, dtype=np.float32).reshape(HID)
    b2 = np.asarray(b2, dtype=np.float32).reshape(1)
    b1r = np.ascontiguousarray(
        np.tile(b1.astype(np.float16), 32)[None, :])
    w2rep = np.ascontiguousarray(
        np.broadcast_to(np.tile(W2, 32)[None, :], (128, 32 * HID)))
    b2col = np.ascontiguousarray(np.broadcast_to(b2[None, :], (128, 1)))
    feat16 = np.ascontiguousarray(
        feat.reshape(T * B, FEAT)).astype(np.float16)
    r2 = r[:, :, 0]
    in_maps = []
    BL = 4096  # transpose block: 1 MB input window, L2-resident
    for c in range(NCORES):
        base = c * TLOC * B
        featT = np.empty((FEAT, TLOC * B), np.float16)
        for j in range(0, TLOC * B, BL):
            featT[:, j:j + BL] = feat16[base + j:base + j + BL, :].T
        rt = np.ascontiguousarray(
            r2[c * TLOC:(c + 1) * TLOC, :].T).reshape(NH, 128, TLOC)
        in_maps.append({
            "feat": featT,
            "rt": rt,
            "w1": W1, "b1r": b1r, "w2rep": w2rep, "b2col": b2col,
        })
    return in_maps


def kernel(r, feat, W1, b1, W2, b2, _run_kwargs=None, _return_results=False):
    from concourse.bass_utils import run_bass_kernel_spmd

    r = np.asarray(r, dtype=np.float32)
    feat = np.asarray(feat, dtype=np.float32)

    nc = _get_program()
    in_maps = _host_in_maps(r, feat, W1, b1, W2, b2)

    kw = _run_kwargs or {}
    res = run_bass_kernel_spmd(nc, in_maps, core_ids=list(range(NCORES)), **kw)

    # host stitch: y = z + P*carry per slab, carry chain across slabs
    y = np.empty((T, B), dtype=np.float32)
    carry = r[0, :, 0].astype(np.float32)
    for c in range(NCORES):
        zc = res.results[c]["z"].transpose(2, 0, 1).reshape(TLOC, B)
        pc = res.results[c]["p"].transpose(2, 0, 1).reshape(TLOC, B)
        y_slab = zc + pc * carry[None, :]
        carry = y_slab[-1]
        y[c * TLOC:(c + 1) * TLOC] = y_slab
    out = y[:, :, None]
    if _return_results:
        return out, res
    return out


# revision 8
# speedup vs baseline: 1.1658x; 1.1658x over previous
"""EMA head kernel for Trainium2 (Bass/Tile), 8 NeuronCores.

Problem: alpha = clip(sigmoid(MLP(feat)), 0.01, 0.99) per (t, b);
         y[0] = r[0]; y[t] = (1-alpha[t])*y[t-1] + alpha[t]*r[t].

Sharding: time dim T=4096 split into 8 slabs of 512 (all B=256 per core).
Each core computes, for its slab, the local affine-scan pieces
    z[t] = A[t]*z[t-1] + Bv[t]   (z[-1] = 0),   A = 1-alpha, Bv = alpha*r
    P[t] = A[t]*P[t-1]           (P[-1] = 1)
and the host stitches slabs with   y = z + P * carry,  carry' = y[-1].
carry_0 = r[0] reproduces y[0] = r[0] exactly: a*r + (1-a)*r = r.

v9: host casts feat to fp16 and pre-transposes to [f, t*b]; the device
streams it with plain contiguous 4MB HWDGE DMAs (~420 GB/s) and runs one
128x128 matmul per (t, b-half) chunk against W1 (rank-1 PE matmuls fold
b1 into each PSUM bank).  h is collected 64 t-steps per PSUM tile (2
banks), so the epilogue is 1 relu (ACT) + 1 mul + 1 reduce (DVE) per
chunk per half.  The alpha->A/Bv->scan tail runs in 2 segments: the
first on GpSimd (own queue; cannot clog DVE's 8-deep FIFO and stall
PSUM recycling), the last on DVE at the end.  z/P leave per segment.
"""

import numpy as np

T, B, FEAT, HID = 4096, 256, 128, 16
NCORES = 8
TLOC = T // NCORES  # 512
NH = 2              # batch halves of 128
CH = 16384          # (t,b) columns per feat chunk (64 t-steps)
NCHUNK = TLOC * B // CH  # 8
TCH = CH // B       # 64 t-steps per chunk

_CACHE = {}


def _build_program():
    import concourse.bacc as bacc
    import concourse.bass as bass
    import concourse.tile as tile
    from concourse import mybir

    fp32 = mybir.dt.float32
    fp16 = mybir.dt.float16
    AF = mybir.ActivationFunctionType
    OP = mybir.AluOpType

    nc = bacc.Bacc("TRN2", target_bir_lowering=False, debug=False,
                   num_devices=NCORES)

    feat_d = nc.dram_tensor("feat", [FEAT, TLOC * B], fp16, kind="ExternalInput")
    rt_d = nc.dram_tensor("rt", [NH, 128, TLOC], fp32, kind="ExternalInput")
    w1_d = nc.dram_tensor("w1", [FEAT, HID], fp16, kind="ExternalInput")
    b1r_d = nc.dram_tensor("b1r", [1, 32 * HID], fp16, kind="ExternalInput")
    w2rep_d = nc.dram_tensor("w2rep", [128, TCH * HID], fp32,
                             kind="ExternalInput")
    b2col_d = nc.dram_tensor("b2col", [128, 1], fp32, kind="ExternalInput")
    z_d = nc.dram_tensor("z", [NH, 128, TLOC], fp32, kind="ExternalOutput")
    p_d = nc.dram_tensor("p", [NH, 128, TLOC], fp32, kind="ExternalOutput")

    with tile.TileContext(nc) as tc:
        with (
            tc.tile_pool(name="singles", bufs=1) as singles,
            tc.tile_pool(name="featin", bufs=3) as featin,
            tc.tile_pool(name="hps", bufs=2, space="PSUM") as hps,
            tc.tile_pool(name="hwork", bufs=2) as hwork,
        ):
            # ------------- constants / small inputs (all HWDGE) -------------
            w1_sb = singles.tile([128, HID], fp16)
            nc.scalar.dma_start(w1_sb, w1_d[:, :])
            b1row = singles.tile([1, 32 * HID], fp16)
            nc.scalar.dma_start(b1row, b1r_d[:, :])
            ones1 = singles.tile([1, 128], fp16)
            nc.vector.memset(ones1, 1.0)
            w2rep = singles.tile([128, TCH, HID], fp32)
            nc.scalar.dma_start(
                w2rep, w2rep_d[:, :].rearrange("p (t h) -> p t h", h=HID))
            b2col = singles.tile([128, 1], fp32)
            nc.scalar.dma_start(b2col, b2col_d[:, :])
            ones_sb = singles.tile([128, TLOC], fp32)
            nc.vector.memset(ones_sb, 1.0)

            rT = [singles.tile([128, TLOC], fp32, tag=f"rT{h}", name=f"rT{h}")
                  for h in range(NH)]
            for h in range(NH):
                nc.scalar.dma_start(rT[h], rt_d[h])

            # per-half alpha_pre accumulators [128 b, t] + tail tiles
            apre = [singles.tile([128, TLOC], fp32, tag=f"apre{h}",
                                 name=f"apre{h}") for h in range(NH)]
            alpha = [singles.tile([128, TLOC], fp32, tag=f"alpha{h}",
                                  name=f"alpha{h}") for h in range(NH)]
            A_sb = [singles.tile([128, TLOC], fp32, tag=f"A{h}",
                                 name=f"A{h}") for h in range(NH)]
            Bv = [singles.tile([128, TLOC], fp32, tag=f"Bv{h}",
                               name=f"Bv{h}") for h in range(NH)]
            z_sb = [singles.tile([128, TLOC], fp32, tag=f"z{h}",
                                 name=f"z{h}") for h in range(NH)]
            p_sb = [singles.tile([128, TLOC], fp32, tag=f"p{h}",
                                 name=f"p{h}") for h in range(NH)]

            def tail_segment(seg, lo, hi, eng):
                """alpha -> clip -> A, Bv -> chained scans on [lo, hi) cols."""
                sl = slice(lo, hi)
                for h in range(NH):
                    nc.scalar.activation(alpha[h][:, sl], apre[h][:, sl],
                                         AF.Sigmoid, bias=b2col)
                    eng.tensor_scalar(alpha[h][:, sl], alpha[h][:, sl],
                                      0.01, 0.99, op0=OP.max, op1=OP.min)
                    eng.tensor_scalar(A_sb[h][:, sl], alpha[h][:, sl],
                                      -1.0, 1.0, op0=OP.mult, op1=OP.add)
                    eng.tensor_mul(Bv[h][:, sl], alpha[h][:, sl], rT[h][:, sl])
                    eng.tensor_tensor_scan(
                        z_sb[h][:, sl], A_sb[h][:, sl], Bv[h][:, sl],
                        0.0 if lo == 0 else z_sb[h][:, lo - 1:lo],
                        op0=OP.mult, op1=OP.add)
                    eng.tensor_tensor_scan(
                        p_sb[h][:, sl], A_sb[h][:, sl], ones_sb[:, sl],
                        1.0 if lo == 0 else p_sb[h][:, lo - 1:lo],
                        op0=OP.mult, op1=OP.mult)
                    nc.scalar.dma_start(z_d[h, :, sl], z_sb[h][:, sl])
                    nc.scalar.dma_start(p_d[h, :, sl], p_sb[h][:, sl])

            # ---------------- main feat pipeline ----------------
            for k in range(NCHUNK):
                ft = featin.tile([128, CH], fp16, tag="ft")
                nc.sync.dma_start(ft, feat_d[:, k * CH:(k + 1) * CH])
                hbank = [hps.tile([128, TCH, HID], fp32, tag=f"h{h}",
                                  name=f"hbank{h}") for h in range(NH)]
                for h in range(NH):
                    # rank-1 bias: psum[:, t, hid] = b1[hid] (one per bank)
                    for q in range(2):
                        nc.tensor.matmul(
                            hbank[h][:, q * 32:(q + 1) * 32, :], ones1, b1row,
                            start=True, stop=False, skip_group_check=True)
                for tt in range(TCH):
                    col = tt * B
                    for h in range(NH):
                        nc.tensor.matmul(
                            hbank[h][:, tt, :],
                            ft[:, col + h * 128:col + (h + 1) * 128],
                            w1_sb, start=False, stop=True,
                            skip_group_check=True)
                t0 = k * TCH
                for h in range(NH):
                    hrelu = hwork.tile([128, TCH, HID], fp32, tag="hrelu")
                    nc.scalar.activation(hrelu, hbank[h], AF.Relu)
                    hw = hwork.tile([128, TCH, HID], fp32, tag="hw")
                    nc.vector.tensor_mul(hw, hrelu, w2rep)
                    nc.vector.tensor_reduce(
                        apre[h][:, t0:t0 + TCH],
                        hw, axis=mybir.AxisListType.X, op=OP.add)
                if k == NCHUNK // 2 - 1:
                    tail_segment(0, 0, TLOC // 2, nc.vector)
            tail_segment(1, TLOC // 2, TLOC, nc.vector)

    nc.finalize()
    return nc


def _get_program():
    if "nc" not in _CACHE:
        _CACHE["nc"] = _build_program()
    return _CACHE["nc"]


def _host_in_maps(r, feat, W1, b1, W2, b2):
    W1 = np.asarray(W1, dtype=np.float16)
    b1 = np.asarray(b1, dtype=np.float32).reshape(HID)
    W2 = np.asarray(W2, dtype=np.float32).reshape(HID)
    b2 = np.asarray(b2, dtype=np.float32).reshape(1)
    b1r = np.ascontiguousarray(
        np.tile(b1.astype(np.float16), 32)[None, :])
    w2rep = np.ascontiguousarray(
        np.broadcast_to(np.tile(W2, TCH)[None, :], (128, TCH * HID)))
    b2col = np.ascontiguousarray(np.broadcast_to(b2[None, :], (128, 1)))
    feat16 = np.ascontiguousarray(
        feat.reshape(T * B, FEAT)).astype(np.float16)
    r2 = r[:, :, 0]
    in_maps = []
    BL = 4096  # transpose block: 1 MB input window, L2-resident
    for c in range(NCORES):
        base = c * TLOC * B
        featT = np.empty((FEAT, TLOC * B), np.float16)
        for j in range(0, TLOC * B, BL):
            featT[:, j:j + BL] = feat16[base + j:base + j + BL, :].T
        rt = np.ascontiguousarray(
            r2[c * TLOC:(c + 1) * TLOC, :].T).reshape(NH, 128, TLOC)
        in_maps.append({
            "feat": featT,
            "rt": rt,
            "w1": W1, "b1r": b1r, "w2rep": w2rep, "b2col": b2col,
        })
    return in_maps


def kernel(r, feat, W1, b1, W2, b2, _run_kwargs=None, _return_results=False):
    from concourse.bass_utils import run_bass_kernel_spmd

    r = np.asarray(r, dtype=np.float32)
    feat = np.asarray(feat, dtype=np.float32)

    nc = _get_program()
    in_maps = _host_in_maps(r, feat, W1, b1, W2, b2)

    kw = _run_kwargs or {}
    res = run_bass_kernel_spmd(nc, in_maps, core_ids=list(range(NCORES)), **kw)

    # host stitch: y = z + P*carry per slab, carry chain across slabs
    y = np.empty((T, B), dtype=np.float32)
    carry = r[0, :, 0].astype(np.float32)
    for c in range(NCORES):
        zc = res.results[c]["z"].transpose(2, 0, 1).reshape(TLOC, B)
        pc = res.results[c]["p"].transpose(2, 0, 1).reshape(TLOC, B)
        y_slab = zc + pc * carry[None, :]
        carry = y_slab[-1]
        y[c * TLOC:(c + 1) * TLOC] = y_slab
    out = y[:, :, None]
    if _return_results:
        return out, res
    return out


# revision 9
# speedup vs baseline: 1.5400x; 1.3210x over previous
"""EMA head kernel for Trainium2 (Bass/Tile), 8 NeuronCores.

Problem: alpha = clip(sigmoid(MLP(feat)), 0.01, 0.99) per (t, b);
         y[0] = r[0]; y[t] = (1-alpha[t])*y[t-1] + alpha[t]*r[t].

Sharding: time dim T=4096 split into 8 slabs of 512 (all B=256 per core).
Each core computes, for its slab, the local affine-scan pieces
    z[t] = A[t]*z[t-1] + Bv[t]   (z[-1] = 0),   A = 1-alpha, Bv = alpha*r
    P[t] = A[t]*P[t-1]           (P[-1] = 1)
and the host stitches slabs with   y = z + P * carry,  carry' = y[-1].
carry_0 = r[0] reproduces y[0] = r[0] exactly: a*r + (1-a)*r = r.

v10: fp8(e4m3) feat/W1; host casts and pre-transposes to [f, t*b]; the device
streams it with plain contiguous 4MB HWDGE DMAs (~420 GB/s) and runs one
128x128 matmul per (t, b-half) chunk against W1 (rank-1 PE matmuls fold
b1 into each PSUM bank).  h is collected 64 t-steps per PSUM tile (2
banks), so the epilogue is 1 relu (ACT) + 1 mul + 1 reduce (DVE) per
chunk per half.  The alpha->A/Bv->scan tail runs in 2 segments: the
first on GpSimd (own queue; cannot clog DVE's 8-deep FIFO and stall
PSUM recycling), the last on DVE at the end.  z/P leave per segment.
"""

import numpy as np

T, B, FEAT, HID = 4096, 256, 128, 16
NCORES = 8
TLOC = T // NCORES  # 512
NH = 2              # batch halves of 128
CH = 16384          # (t,b) columns per feat chunk (64 t-steps)
NCHUNK = TLOC * B // CH  # 8
TCH = CH // B       # 64 t-steps per chunk

_CACHE = {}


def _build_program():
    import concourse.bacc as bacc
    import concourse.bass as bass
    import concourse.tile as tile
    from concourse import mybir

    fp32 = mybir.dt.float32
    fp16 = mybir.dt.float16
    fp8 = mybir.dt.float8e4
    AF = mybir.ActivationFunctionType
    OP = mybir.AluOpType

    nc = bacc.Bacc("TRN2", target_bir_lowering=False, debug=False,
                   num_devices=NCORES)

    feat_d = nc.dram_tensor("feat", [FEAT, TLOC * B], fp8, kind="ExternalInput")
    rt_d = nc.dram_tensor("rt", [NH, 128, TLOC], fp32, kind="ExternalInput")
    w1_d = nc.dram_tensor("w1", [FEAT, HID], fp8, kind="ExternalInput")
    b1r_d = nc.dram_tensor("b1r", [1, 32 * HID], fp16, kind="ExternalInput")
    w2rep_d = nc.dram_tensor("w2rep", [128, TCH * HID], fp32,
                             kind="ExternalInput")
    b2col_d = nc.dram_tensor("b2col", [128, 1], fp32, kind="ExternalInput")
    z_d = nc.dram_tensor("z", [NH, 128, TLOC], fp32, kind="ExternalOutput")
    p_d = nc.dram_tensor("p", [NH, 128, TLOC], fp32, kind="ExternalOutput")

    with tile.TileContext(nc) as tc:
        with (
            tc.tile_pool(name="singles", bufs=1) as singles,
            tc.tile_pool(name="featin", bufs=3) as featin,
            tc.tile_pool(name="hps", bufs=2, space="PSUM") as hps,
            tc.tile_pool(name="hwork", bufs=2) as hwork,
        ):
            # ------------- constants / small inputs (all HWDGE) -------------
            w1_sb = singles.tile([128, HID], fp8)
            nc.scalar.dma_start(w1_sb, w1_d[:, :])
            b1row = singles.tile([1, 32 * HID], fp16)
            nc.scalar.dma_start(b1row, b1r_d[:, :])
            ones1 = singles.tile([1, 128], fp16)
            nc.vector.memset(ones1, 1.0)
            w2rep = singles.tile([128, TCH, HID], fp32)
            nc.scalar.dma_start(
                w2rep, w2rep_d[:, :].rearrange("p (t h) -> p t h", h=HID))
            b2col = singles.tile([128, 1], fp32)
            nc.scalar.dma_start(b2col, b2col_d[:, :])
            ones_sb = singles.tile([128, TLOC], fp32)
            nc.vector.memset(ones_sb, 1.0)

            rT = [singles.tile([128, TLOC], fp32, tag=f"rT{h}", name=f"rT{h}")
                  for h in range(NH)]
            for h in range(NH):
                nc.scalar.dma_start(rT[h], rt_d[h])

            # per-half alpha_pre accumulators [128 b, t] + tail tiles
            apre = [singles.tile([128, TLOC], fp32, tag=f"apre{h}",
                                 name=f"apre{h}") for h in range(NH)]
            alpha = [singles.tile([128, TLOC], fp32, tag=f"alpha{h}",
                                  name=f"alpha{h}") for h in range(NH)]
            A_sb = [singles.tile([128, TLOC], fp32, tag=f"A{h}",
                                 name=f"A{h}") for h in range(NH)]
            Bv = [singles.tile([128, TLOC], fp32, tag=f"Bv{h}",
                               name=f"Bv{h}") for h in range(NH)]
            z_sb = [singles.tile([128, TLOC], fp32, tag=f"z{h}",
                                 name=f"z{h}") for h in range(NH)]
            p_sb = [singles.tile([128, TLOC], fp32, tag=f"p{h}",
                                 name=f"p{h}") for h in range(NH)]

            def tail_segment(seg, lo, hi, eng):
                """alpha -> clip -> A, Bv -> chained scans on [lo, hi) cols."""
                sl = slice(lo, hi)
                for h in range(NH):
                    nc.scalar.activation(alpha[h][:, sl], apre[h][:, sl],
                                         AF.Sigmoid, bias=b2col)
                    eng.tensor_scalar(alpha[h][:, sl], alpha[h][:, sl],
                                      0.01, 0.99, op0=OP.max, op1=OP.min)
                    eng.tensor_scalar(A_sb[h][:, sl], alpha[h][:, sl],
                                      -1.0, 1.0, op0=OP.mult, op1=OP.add)
                    eng.tensor_mul(Bv[h][:, sl], alpha[h][:, sl], rT[h][:, sl])
                    eng.tensor_tensor_scan(
                        z_sb[h][:, sl], A_sb[h][:, sl], Bv[h][:, sl],
                        0.0 if lo == 0 else z_sb[h][:, lo - 1:lo],
                        op0=OP.mult, op1=OP.add)
                    eng.tensor_tensor_scan(
                        p_sb[h][:, sl], A_sb[h][:, sl], ones_sb[:, sl],
                        1.0 if lo == 0 else p_sb[h][:, lo - 1:lo],
                        op0=OP.mult, op1=OP.mult)
                    nc.scalar.dma_start(z_d[h, :, sl], z_sb[h][:, sl])
                    nc.scalar.dma_start(p_d[h, :, sl], p_sb[h][:, sl])

            # ---------------- main feat pipeline ----------------
            for k in range(NCHUNK):
                ft = featin.tile([128, CH], fp8, tag="ft")
                if k == NCHUNK - 1:
                    for q in range(2):
                        nc.sync.dma_start(
                            ft[:, q * (CH // 2):(q + 1) * (CH // 2)],
                            feat_d[:, k * CH + q * (CH // 2):
                                   k * CH + (q + 1) * (CH // 2)])
                else:
                    nc.sync.dma_start(ft, feat_d[:, k * CH:(k + 1) * CH])
                hbank = [hps.tile([128, TCH, HID], fp32, tag=f"h{h}",
                                  name=f"hbank{h}") for h in range(NH)]
                for h in range(NH):
                    # rank-1 bias: psum[:, t, hid] = b1[hid] (one per bank)
                    for q in range(2):
                        nc.tensor.matmul(
                            hbank[h][:, q * 32:(q + 1) * 32, :], ones1, b1row,
                            start=True, stop=False, skip_group_check=True)
                for tt in range(TCH):
                    col = tt * B
                    for h in range(NH):
                        nc.tensor.matmul(
                            hbank[h][:, tt, :],
                            ft[:, col + h * 128:col + (h + 1) * 128],
                            w1_sb, start=False, stop=True,
                            skip_group_check=True)
                t0 = k * TCH
                for h in range(NH):
                    hrelu = hwork.tile([128, TCH, HID], fp32, tag="hrelu")
                    nc.scalar.activation(hrelu, hbank[h], AF.Relu)
                    hw = hwork.tile([128, TCH, HID], fp32, tag="hw")
                    nc.vector.tensor_mul(hw, hrelu, w2rep)
                    nc.vector.tensor_reduce(
                        apre[h][:, t0:t0 + TCH],
                        hw, axis=mybir.AxisListType.X, op=OP.add)
                if k == NCHUNK // 2 - 1:
                    tail_segment(0, 0, TLOC // 2, nc.vector)
                elif k == NCHUNK - 3:
                    tail_segment(1, TLOC // 2, 3 * TLOC // 4, nc.vector)
            tail_segment(2, 3 * TLOC // 4, TLOC, nc.vector)

    nc.finalize()
    return nc


def _get_program():
    if "nc" not in _CACHE:
        _CACHE["nc"] = _build_program()
    return _CACHE["nc"]


def _host_in_maps(r, feat, W1, b1, W2, b2):
    import ml_dtypes
    W1 = np.asarray(W1, dtype=np.float32).astype(ml_dtypes.float8_e4m3)
    b1 = np.asarray(b1, dtype=np.float32).reshape(HID)
    W2 = np.asarray(W2, dtype=np.float32).reshape(HID)
    b2 = np.asarray(b2, dtype=np.float32).reshape(1)
    b1r = np.ascontiguousarray(
        np.tile(b1.astype(np.float16), 32)[None, :])
    w2rep = np.ascontiguousarray(
        np.broadcast_to(np.tile(W2, TCH)[None, :], (128, TCH * HID)))
    b2col = np.ascontiguousarray(np.broadcast_to(b2[None, :], (128, 1)))
    feat16 = np.ascontiguousarray(
        feat.reshape(T * B, FEAT)).astype(np.float16)
    r2 = r[:, :, 0]
    in_maps = []
    BL = 4096  # transpose block: 1 MB input window, L2-resident
    for c in range(NCORES):
        base = c * TLOC * B
        featT = np.empty((FEAT, TLOC * B), np.float16)
        for j in range(0, TLOC * B, BL):
            featT[:, j:j + BL] = feat16[base + j:base + j + BL, :].T
        featT = featT.astype(ml_dtypes.float8_e4m3)
        rt = np.ascontiguousarray(
            r2[c * TLOC:(c + 1) * TLOC, :].T).reshape(NH, 128, TLOC)
        in_maps.append({
            "feat": featT,
            "rt": rt,
            "w1": W1, "b1r": b1r, "w2rep": w2rep, "b2col": b2col,
        })
    return in_maps


def kernel(r, feat, W1, b1, W2, b2, _run_kwargs=None, _return_results=False):
    from concourse.bass_utils import run_bass_kernel_spmd

    r = np.asarray(r, dtype=np.float32)
    feat = np.asarray(feat, dtype=np.float32)

    nc = _get_program()
    in_maps = _host_in_maps(r, feat, W1, b1, W2, b2)

    kw = _run_kwargs or {}
    res = run_bass_kernel_spmd(nc, in_maps, core_ids=list(range(NCORES)), **kw)

    # host stitch: y = z + P*carry per slab, carry chain across slabs
    y = np.empty((T, B), dtype=np.float32)
    carry = r[0, :, 0].astype(np.float32)
    for c in range(NCORES):
        zc = res.results[c]["z"].transpose(2, 0, 1).reshape(TLOC, B)
        pc = res.results[c]["p"].transpose(2, 0, 1).reshape(TLOC, B)
        y_slab = zc + pc * carry[None, :]
        carry = y_slab[-1]
        y[c * TLOC:(c + 1) * TLOC] = y_slab
    out = y[:, :, None]
    if _return_results:
        return out, res
    return out
